# revision 55
# baseline (speedup 1.0000x reference)
"""Trainium2 Bass kernel for Swin-style multi-head attention.

Problem: x[128,197,768] -> qkv -> 12-head attention with relative-position
bias -> proj. Data-parallel over batch across 8 NeuronCores (16 batches/core).

Dataflow (per core):
  - x, qkv_w, proj_w, out move in bf16 (halves HBM traffic; f32 only in
    PSUM and for the bias factors). Host converts and pads x to 198 cols
    so every bf16 row stride stays 4B-aligned and fill matmuls stream one
    CONTIGUOUS [2,198] moving block per ct chunk (full rate, single
    ldweights per chunk).
  - Weights/bias load in CHUNKS across three DMA queues (SP / gpsimd /
    ACT; a DMA occupies its queue for the whole transfer) in need order;
    projw is sprinkled one chunk per pair-0 slot. First matmul starts
    ~4us in instead of ~42us.
  - q,k feature-major bf16 in PER-FT tiles: the fill's ACT drain writes
    each [128, 2, 198] tile contiguously (legal bf16 ACT write), so no
    gpsimd placement hop sits on the fill->scores chain -- worth ~2x on
    HW. v token-major in the augmented layout [t, 12, 66-alloc/65-used]
    whose 65th column is ones so the AV matmul emits softmax row-sums for
    free (ACT stages the v drain, gpsimd places it -- moving that drain
    to DVE was tried and REGRESSED: DVE is the binding epilogue engine).
    scores/AV bf16 with unpadded n=197.
  - softmax without max-subtraction, bias pre-gathered/pre-exponentiated
    on host; per-head exps (ACT) -> one DVE multiply per m-tile -> bf16
    numerators; normalization via DVE reciprocal + GPSIMD
    partition_broadcast + DVE multiply into persistent zeroed o_all
    tiles; proj bf16 + DVE bias add.

Schedule: one continuous slot pipeline. Each pair's 12 attention units
interleave with that pair's OWN qk fills (slots 0-5), b0/b1 vfills, and
the PREVIOUS pair's 6 projection units, so PE never runs a bare fill
block and unit epilogues (exp->mult->AV, ~1.5us) hide behind filler
work. AV lags scores by pdepth=4 (2 on the last pair to shrink the
drain). x is prefetched a pair ahead (double-buffered).

HW lessons baked in (found the hard way; the CoreSim cost model does not
price these):
  - Only ONE accumulation chain per PSUM bank at a time: interleaved or
    back-to-back singleton chains into one bank corrupt partial sums or
    fault. Per-head score matmuls therefore target separate banks.
  - gpsimd elementwise (Q7 software loops) is ~40x the modeled cost --
    keep gpsimd to tensor_copy/partition_broadcast only.
  - ACT Identity and Exp live in different activation tables; mixing
    them thrashes ACT_TABLE_LOAD (1.3us each). Proj bias lives on DVE.
  - Multi-dim strided moving operands stream below full rate; keep
    moving blocks contiguous (pad-and-cover instead of stride-and-skip).
PSUM budget: 2 fill/proj banks + 4 score banks + 2 AV banks = 8.
"""

import sys

import numpy as np

for _p in ('/opt/trn_rl_repo', '/root/.axon_site/_ro/trn_rl_repo'):
    if _p not in sys.path:
        sys.path.insert(0, _p)

B = 128
N = 197
NPAD = 256
C = 768
H = 12
DH = 64
SCALE = DH ** -0.5
NCORES = 8
BLOC = B // NCORES  # 16
M0, M1 = 128, N - 128  # key-dim tiles: 128 + 69


def build_nc(b_loc=BLOC, pdepth=4, reps=1, gps_mt1=False, act_projb=False,
             pup_bufs=5, psbig_bufs=2, pss_bufs=2, pso_bufs=2, sap_bufs=2,
             par_dma=True):
    """Build the per-core Bass program."""
    import concourse.bacc as bacc
    import concourse.tile as tile
    from concourse import library_config, mybir

    f32 = mybir.dt.float32
    abf = mybir.dt.bfloat16
    N2 = N + 1  # 198: bf16 tiles padded so all strides stay 4B-aligned

    nc = bacc.Bacc("TRN2", target_bir_lowering=False, debug=False)
    xT = nc.dram_tensor("xT", [b_loc, C, N2], abf, kind="ExternalInput").ap()
    qkv_wT = nc.dram_tensor("qkv_wT", [C, 3 * C], abf,
                            kind="ExternalInput").ap()
    proj_wT = nc.dram_tensor("proj_wT", [C, C], abf,
                             kind="ExternalInput").ap()
    proj_bt = nc.dram_tensor("proj_bt", [128, 6], f32, kind="ExternalInput").ap()
    biasT = nc.dram_tensor("biasT", [H, N, NPAD], f32, kind="ExternalInput").ap()
    outT = nc.dram_tensor("outT", [b_loc, C, N], abf, kind="ExternalOutput").ap()

    n_pairs = b_loc // 2

    with tile.TileContext(nc) as tc:
        with (
            tc.tile_pool(name="consts", bufs=1) as consts,
            tc.tile_pool(name="xtp", bufs=2) as xtp,
            tc.tile_pool(name="qkp", bufs=2) as qkp,
            tc.tile_pool(name="vtp", bufs=2) as vtp,
            tc.tile_pool(name="sap", bufs=sap_bufs) as sap,
            tc.tile_pool(name="pup", bufs=pup_bufs) as pup,
            tc.tile_pool(name="recp", bufs=2) as recp,
            tc.tile_pool(name="oallp", bufs=2) as oallp,
            tc.tile_pool(name="obp", bufs=3) as obp,
            tc.tile_pool(name="psbig", bufs=psbig_bufs, space="PSUM") as psbig,
            tc.tile_pool(name="pss", bufs=pss_bufs, space="PSUM") as pss,
            tc.tile_pool(name="pso", bufs=pso_bufs, space="PSUM") as pso,
        ):
            nc.gpsimd.load_library(library_config.attnmlp)

            # chunked weight tiles: 12 q/k f-tiles + 2 v halves
            qkw = [consts.tile([128, 6, 128], abf, name=f"qkw{ft}")
                   for ft in range(12)]
            vw = [consts.tile([128, 6, 384], abf, name=f"vw{half}")
                  for half in (0, 1)]
            projw_sb = consts.tile([128, 6, C], abf)
            projb_sb = consts.tile([128, 6], f32)
            bias0_sb = consts.tile([128, H, N], f32)
            bias1_sb = consts.tile([128, H, N], f32)
            bias_sb = (bias0_sb, bias1_sb)
            ones_sb = consts.tile([128, H], abf)

            qkvw_r = qkv_wT.rearrange("(ct p) f -> p ct f", p=128)
            projw_r = proj_wT.rearrange("(hp p) e -> p hp e", p=128)

            # two DMA queues: weights stream on SP while x/bias stream on
            # the gpsimd queue, so the prologue is not DMA-serialized.
            xt_eng = nc.gpsimd if par_dma else nc.sync

            def dma_xt(pp):
                # pair 0: split halves across gpsimd+SP (both queues idle);
                # later pairs: both halves on SP (a DMA occupies its queue
                # for the whole transfer, and gpsimd has compute mid-pair)
                b0_ = 2 * (pp % n_pairs)
                xt = xtp.tile([128, 6, 2, N2], abf, tag="xt",
                              name=f"xt{pp}")
                engs = (xt_eng, nc.sync) if pp == 0 else (nc.sync, nc.sync)
                for b in (0, 1):
                    engs[b].dma_start(
                        xt[:, :, b, :],
                        xT[b0_ + b].rearrange("(ct p) n -> p ct n", p=128),
                    )
                return xt

            def load_consts():
                # A DMA occupies its issuing queue for the whole transfer.
                # Prologue streams: ACT carries bias0 (ACT idles until the
                # first qst copy); gpsimd carries qkw6-11 + vw1 (its copies
                # start later than SP's pressure allows); SP carries the
                # rest, in need order. projw/projb are sprinkled in chunks
                # across pair-0 slots (see make_sprinkle).
                bias_eng = nc.scalar if par_dma else nc.sync
                gp_eng = xt_eng if par_dma else nc.sync

                def dma_qkw(ft):
                    eng = gp_eng if ft >= 6 else nc.sync
                    eng.dma_start(
                        qkw[ft], qkvw_r[:, :, ft * 128:(ft + 1) * 128])

                def dma_vw(half, eng):
                    eng.dma_start(
                        vw[half],
                        qkvw_r[:, :, 2 * C + half * 384:2 * C + (half + 1) * 384])

                for ft in (0, 1, 2, 6, 7, 8):
                    dma_qkw(ft)
                biasT0 = biasT[:, 0:M0, :N].rearrange("h p n -> p h n")
                bias_eng.dma_start(bias0_sb[:, :6], biasT0[:, :6])
                bias_eng.dma_start(bias0_sb[:, 6:], biasT0[:, 6:])
                dma_vw(1, gp_eng)
                for ft in (3, 4, 5, 9, 10, 11):
                    dma_qkw(ft)
                dma_vw(0, nc.sync)
                biasT1 = biasT[:, M0:N, :N].rearrange("h p n -> p h n")
                nc.sync.dma_start(bias1_sb[:M1, :6], biasT1[:, :6])
                nc.sync.dma_start(bias1_sb[:M1, 6:], biasT1[:, 6:])
                nc.vector.memset(ones_sb, 1.0)

            def make_sprinkle():
                # late consts, chunked small and issued one per pair-0 slot
                # inside engine idle windows
                out = []
                for e6 in range(6):
                    out.append(lambda e6=e6: xt_eng.dma_start(
                        projw_sb[:, :, e6 * 128:(e6 + 1) * 128],
                        projw_r[:, :, e6 * 128:(e6 + 1) * 128]))
                out.append(lambda: nc.sync.dma_start(projb_sb, proj_bt))
                return out

            def make_qk_fill(qk, xt, ft):
                def fill():
                    # single chain, moving covers the padding so the AP is
                    # CONTIGUOUS (full-rate streaming, one ldweights per ct);
                    # junk cols 197/395 are never read downstream. The ACT
                    # drain writes the per-ft tile CONTIGUOUSLY (legal bf16
                    # ACT write), so no gpsimd placement hop is needed.
                    ps = psbig.tile([128, 2, N2], f32, tag="mmbig")
                    for ct in range(6):
                        nc.tensor.matmul(
                            ps,
                            qkw[ft][:, ct, :],
                            xt[:, ct],
                            start=(ct == 0),
                            stop=(ct == 5),
                        )
                    nc.scalar.copy(out=qk[ft], in_=ps)
                return fill

            def alloc_vts(pp):
                # 66-col alloc keeps bf16 head-stride 4B-aligned;
                # AV reads cols 0:65 only.
                return [[vtp.tile([128, H, 66], abf, tag=f"vt{b}{tci}",
                                  name=f"vt{b}{tci}_{pp}")
                         for tci in (0, 1)] for b in (0, 1)]

            def make_vfill(vts, xt, b, tci, half):
                def vfill():
                    t0, tsz = ((0, M0), (M0, M1))[tci]
                    vt = vts[b][tci]
                    vt_r = vt.rearrange("p (g two) c -> p two g c", two=2)
                    psv = psbig.tile([128, 384], f32, tag="mmbig")
                    for ct in range(6):
                        nc.tensor.matmul(
                            psv[:tsz],
                            xt[:, ct, b, t0:t0 + tsz],
                            vw[half][:, ct, :],
                            start=(ct == 0),
                            stop=(ct == 5),
                        )
                    vst = obp.tile([128, 384], abf, tag="vst")
                    nc.scalar.copy(out=vst[:tsz], in_=psv[:tsz])
                    vst_r = vst.rearrange("p (g two d) -> p two g d",
                                          two=2, d=64)
                    for par in (0, 1):
                        nc.gpsimd.tensor_copy(
                            out=vt_r[:tsz, par, half * 3:(half + 1) * 3, 0:64],
                            in_=vst_r[:tsz, par],
                        )
                    if half == 1:
                        nc.gpsimd.tensor_copy(out=vt[:tsz, :, 64],
                                              in_=ones_sb[:tsz])
                return vfill

            total = reps * n_pairs
            prev_proj_units = []

            o_alls = [oallp.tile([128, 6, 2, N2], abf, name=f"o_all{i}")
                      for i in (0, 1)]

            xt_cur = dma_xt(0)
            load_consts()
            for t in o_alls:
                nc.vector.memset(t, 0.0)
            sprinkle = make_sprinkle()

            for pp in range(total):
                b0 = 2 * (pp % n_pairs)
                xt = xt_cur
                if pp + 1 < total:
                    xt_next = dma_xt(pp + 1)

                # Per-slot pre-work: each pair fills its OWN qk tile
                # interleaved with its early attention units, so fills,
                # vfills, and the previous pair's projections act as PE
                # filler around every unit's softmax epilogue.
                qk = [qkp.tile([128, 2, N2], abf, tag=f"qk{ft}",
                               name=f"qk{ft}_{pp}") for ft in range(H)]
                vts = alloc_vts(pp)
                pre = [[] for _ in range(12)]
                for hp in range(6):
                    pre[hp] += [make_qk_fill(qk, xt, hp),
                                make_qk_fill(qk, xt, 6 + hp)]
                # b0 vfills at slots 2-4, half-0 first (vw chunks land after
                # qkw in the prologue; AV of unit 0 is emitted at the end of
                # slot 4). The last pair runs a shorter AV lag (pdepth 2) to
                # shrink the final drain, so its b0 vfills move to slots 0-1.
                last = (pp + 1 == total)
                pdepth_eff = 2 if last else pdepth
                vf0_slots = (0, 0, 1, 1) if last else (2, 3, 4, 4)
                for j, (tci, half) in enumerate(
                        (t, h) for h in (0, 1) for t in (0, 1)):
                    pre[vf0_slots[j]] += [make_vfill(vts, xt, 0, tci, half)]
                    pre[5 + j] += [make_vfill(vts, xt, 1, tci, half)]
                for j, u in enumerate(prev_proj_units):
                    pre[6 + j] += [u]

                # ---- attention, software-pipelined over (batch, head-pair) ----
                o_all = o_alls[pp % 2]

                def emit_scores(b, hp):
                    """scores + exp + bias for both heads of pair hp -> pu tiles."""
                    h0 = 2 * hp
                    q0 = qk[hp][0:64, b, :N]
                    k0 = qk[6 + hp][0:64, b, :N]
                    q1 = qk[hp][64:128, b, :N]
                    k1 = qk[6 + hp][64:128, b, :N]
                    pus = []
                    for mt, (m0, msz) in enumerate(((0, M0), (M0, M1))):
                        ps_e = pss.tile([128, NPAD], f32, tag="se")
                        ps_o = pss.tile([128, NPAD], f32, tag="so")
                        nc.tensor.matmul(
                            ps_e[:msz, :N], k0[:, m0:m0 + msz], q0,
                            start=True, stop=True,
                        )
                        nc.tensor.matmul(
                            ps_o[:msz, :N], k1[:, m0:m0 + msz], q1,
                            start=True, stop=True,
                        )
                        sa_pair = sap.tile([128, 2, N], f32, tag="sa")
                        pu_pair = pup.tile([128, 2, N2], abf, tag=f"pu{mt}")
                        nc.scalar.activation(
                            out=sa_pair[:msz, 0, :], in_=ps_e[:msz, :N],
                            func=mybir.ActivationFunctionType.Exp, scale=SCALE,
                        )
                        nc.scalar.activation(
                            out=sa_pair[:msz, 1, :], in_=ps_o[:msz, :N],
                            func=mybir.ActivationFunctionType.Exp, scale=SCALE,
                        )
                        nc.vector.tensor_mul(
                            out=pu_pair[:msz, :, :N], in0=sa_pair[:msz],
                            in1=bias_sb[mt][:msz, h0:h0 + 2, :],
                        )
                        pus.append(pu_pair)
                    return pus

                def emit_av(b, hp, pus):
                    h0, h1 = 2 * hp, 2 * hp + 1
                    vt0, vt1 = vts[b]
                    ps_pair = pso.tile([128, 2, NPAD], f32, tag="opair")
                    for par, h in ((0, h0), (1, h1)):
                        nc.tensor.matmul(
                            ps_pair[0:65, par, :N], vt0[:, h, 0:65],
                            pus[0][:, par, :N], start=True, stop=False,
                        )
                        nc.tensor.matmul(
                            ps_pair[0:65, par, :N], vt1[:M1, h, 0:65],
                            pus[1][:M1, par, :N], start=False, stop=True,
                        )
                    rec_pair = recp.tile([1, 2, N], f32, tag="rec")
                    nc.vector.reciprocal(out=rec_pair,
                                         in_=ps_pair[64:65, :, :N])
                    recb_pair = recp.tile([64, 2, N], f32, tag="recb")
                    nc.gpsimd.partition_broadcast(recb_pair, rec_pair)
                    for par in (0, 1):
                        nc.vector.tensor_mul(
                            out=o_all[par * 64:par * 64 + 64, hp, b, :N],
                            in0=ps_pair[0:64, par, :N],
                            in1=recb_pair[:, par, :],
                        )

                def make_proj_unit(et, o_all_=o_all, b0_=b0,
                                   last_=None):
                    last_ = last if last_ is None else last_
                    def unit():
                        psp = psbig.tile([128, 2, N2], f32, tag="mmbig", name=f"psp{et}")
                        for hp in range(6):
                            nc.tensor.matmul(
                                psp,
                                projw_sb[:, hp, et * 128:(et + 1) * 128],
                                o_all_[:, hp],
                                start=(hp == 0),
                                stop=(hp == 5),
                            )
                        ob = obp.tile([128, 2, N], abf, tag="ob", name=f"ob{et}")
                        # alternate ACT/DVE for the bias add and SP/gpsimd
                        # for the out DMA so the final drain parallelizes
                        if act_projb and et % 2 == 0:
                            nc.scalar.add(ob, psp[:, :, :N],
                                          projb_sb[:, et:et + 1])
                        else:
                            nc.vector.tensor_scalar_add(ob, psp[:, :, :N],
                                                        projb_sb[:, et:et + 1])
                        # the last pair's out-DMAs run in the epilogue with
                        # the gpsimd queue idle: split them across two queues
                        eng = xt_eng if (last_ and et % 2) else nc.sync
                        eng.dma_start(
                            outT[b0_:b0_ + 2, et * 128:(et + 1) * 128, :].rearrange(
                                "b p n -> p b n"
                            ),
                            ob,
                        )
                    return unit

                work = [(b, hp) for b in (0, 1) for hp in range(6)]
                pending = []
                for i, (b, hp) in enumerate(work):
                    for w in pre[i]:
                        w()
                    if sprinkle and i >= 2:
                        sprinkle.pop(0)()
                    pending.append((b, hp, emit_scores(b, hp)))
                    if len(pending) > pdepth_eff:
                        pb, php, ppus = pending.pop(0)
                        emit_av(pb, php, ppus)
                for pb, php, ppus in pending:
                    emit_av(pb, php, ppus)
                prev_proj_units = [make_proj_unit(et) for et in range(6)]
                if pp + 1 < total:
                    xt_cur = xt_next

            # ---- final pair's proj ----
            for u in prev_proj_units:
                u()
    nc.compile()
    return nc


def prep_inputs(x, qkv_w, proj_w, proj_b, bias_table, rel_idx):
    """Host-side data prep shared by kernel() and test harness."""
    import ml_dtypes
    bf16 = np.dtype(ml_dtypes.bfloat16)

    x = np.asarray(x, np.float32)
    qkv_w = np.asarray(qkv_w, np.float32)
    proj_w = np.asarray(proj_w, np.float32)
    proj_b = np.asarray(proj_b, np.float32)
    bias_table = np.asarray(bias_table, np.float32)
    rel_idx = np.asarray(rel_idx)

    xTf = np.zeros((NCORES, BLOC, C, N + 1), np.float32)
    xTf[:, :, :, :N] = x.reshape(NCORES, BLOC, N, C).transpose(0, 1, 3, 2)
    xT = xTf.astype(bf16)
    qkv_wT = np.ascontiguousarray(qkv_w.T).astype(bf16)
    proj_wT = np.ascontiguousarray(proj_w.T).astype(bf16)
    proj_bt = np.ascontiguousarray(proj_b.reshape(6, 128).T)
    bias_full = bias_table[rel_idx]  # [n, m, h]
    biasT = np.ones((H, N, NPAD), np.float32)
    biasT[:, :, :N] = np.exp(bias_full.transpose(2, 1, 0))
    return xT, qkv_wT, proj_wT, proj_bt, biasT


def make_in_maps(x, qkv_w, proj_w, proj_b, bias_table, rel_idx):
    xT, qkv_wT, proj_wT, proj_bt, biasT = prep_inputs(
        x, qkv_w, proj_w, proj_b, bias_table, rel_idx
    )
    return [
        {
            "xT": np.ascontiguousarray(xT[c]),
            "qkv_wT": qkv_wT,
            "proj_wT": proj_wT,
            "proj_bt": proj_bt,
            "biasT": biasT,
        }
        for c in range(NCORES)
    ]


def assemble_out(res):
    outs = np.stack([np.asarray(res.results[c]["outT"], np.float32)
                     for c in range(NCORES)])  # [8,16,768,197]
    out = outs.reshape(B, C, N).transpose(0, 2, 1)
    return np.ascontiguousarray(out, np.float32)


_NC_CACHE = {}


def _get_nc(**kw):
    key = tuple(sorted(kw.items()))
    if key not in _NC_CACHE:
        _NC_CACHE[key] = build_nc(**kw)
    return _NC_CACHE[key]


def kernel(x, qkv_w, proj_w, proj_b, bias_table, rel_idx, _trace=False):
    from concourse.bass_utils import run_bass_kernel_spmd

    in_maps = make_in_maps(x, qkv_w, proj_w, proj_b, bias_table, rel_idx)
    nc = _get_nc()
    res = run_bass_kernel_spmd(nc, in_maps, list(range(NCORES)), trace=_trace)
    out = assemble_out(res)
    if _trace:
        return out, res
    return out
